# revision 1
# baseline (speedup 1.0000x reference)
"""Trainium2 Bass kernel for nn_ContentMultiheadAttention_523986010170.

Full (unsharded) inputs in, full output out. Internally shards across 8
NeuronCores: core c handles batch b = c//2 and query-row half c%2 (1024 of
2048 rows), computing all 8 heads for its slice. Outputs are disjoint
[1024, 512] blocks of the [S, B, E] result, gathered on the host.

v2 design (ACT-paced): the scalar engine's 128 exp instructions are the
hard floor (~141us); everything else is scheduled to keep that stream
gap-free:

  - 2-head groups x 16 t-blocks: one [128,1024] f32 score psum + one exp
    per iteration.  PSUM = 4 banks (scores x2) + 2 (AV accum) + 2 (proj).
  - rowsum folded into the AV matmul: lhsT = [ones | v_h] (M=65), so row 0
    of the AV accumulator is the softmax denominator -- no separate rowsum
    matmuls.  Normalized via reciprocal + gpsimd partition_broadcast.
  - A = exp(scores) * exp(mask): single [128,1024] multiply per iteration
    with the mask factor free-dim-broadcast; split DVE(3/4) / GPSIMD(1/4).
  - exp(mask) prepass interleaved into the ACT stream just-in-time; all
    DMAs chunked and ordered by first use; q/k/v projections drained
    through a per-iteration work plan so the exp stream starts at ~6us.

Host-side work is limited to layout (transpose/slice/concat), the exact
power-of-two weight prescale, and adding out_proj_bias (a zero vector per
the problem spec; in_proj biases are likewise zero and are not applied).
"""

import numpy as np

S, B, E = 2048, 4, 512
H, D = 8, 64
NCORES = 8
SC = S // 2          # query rows per core
T = S                # key rows (full)
NT = T // 128        # t-blocks of 128
KC = E // 128        # contraction chunks for projections
NG = 8               # attention groups: (sc-chunk, head-pair)
AV_LAG = 3           # AV emission lag (iters) behind QK for DVE-mul tiles
GPS_LAG = 7          # lag for gpsimd-mul tiles (hides Q7 latency)
GPS_T = (1, 5, 9)    # iterations whose A-multiply goes to gpsimd

_compiled = None


def _build():
    import concourse.bacc as bacc
    import concourse.mybir as mybir
    import concourse.tile as tile

    f32 = mybir.dt.float32
    bf16 = mybir.dt.bfloat16
    Exp = mybir.ActivationFunctionType.Exp
    Mult = mybir.AluOpType.mult

    nc = bacc.Bacc("TRN2", target_bir_lowering=False, debug=False)

    xq_d = nc.dram_tensor("xq_t", [E, SC], bf16, kind="ExternalInput")
    xk_d = nc.dram_tensor("xk_t", [E, T], bf16, kind="ExternalInput")
    xv_d = nc.dram_tensor("xv_t", [E, T], bf16, kind="ExternalInput")
    mask_d = nc.dram_tensor("mask_t", [T, SC], bf16, kind="ExternalInput")
    wq_d = nc.dram_tensor("wq_t", [E, E], bf16, kind="ExternalInput")
    wk_d = nc.dram_tensor("wk_t", [E, E], bf16, kind="ExternalInput")
    wv_d = nc.dram_tensor("wv_t", [E, E], bf16, kind="ExternalInput")
    wo_d = nc.dram_tensor("wo_t", [E, E], bf16, kind="ExternalInput")
    out_d = nc.dram_tensor("out", [SC, E], f32, kind="ExternalOutput")

    with tile.TileContext(nc) as tc:
        with (
            tc.tile_pool(name="persist", bufs=1) as pp,
            tc.tile_pool(name="mstage", bufs=2) as mst,
            tc.tile_pool(name="et", bufs=5) as etp,
            tc.tile_pool(name="a2", bufs=8) as a2p,
            tc.tile_pool(name="ao", bufs=16) as aop,
            tc.tile_pool(name="norm", bufs=2) as nrm,
            tc.tile_pool(name="osb", bufs=3) as osp,
            tc.tile_pool(name="sp", bufs=2, space="PSUM") as spp,
            tc.tile_pool(name="av", bufs=2, space="PSUM") as avp,
            tc.tile_pool(name="pj", bufs=2, space="PSUM") as pjp,
        ):
            # ---- persistent SBUF tensors ----
            wq = pp.tile([128, KC, E], bf16, tag="wq")
            wk = pp.tile([128, KC, E], bf16, tag="wk")
            wv = pp.tile([128, KC, E], bf16, tag="wv")
            wo = pp.tile([64, H, E], bf16, tag="wo")  # head-major, base-0 rows
            xq = pp.tile([128, KC, SC], bf16, tag="xq")
            xk = pp.tile([128, KC, T], bf16, tag="xk")
            xv = pp.tile([128, KC, T], bf16, tag="xv")
            g = pp.tile([128, NT, SC], bf16, tag="g")
            qT = pp.tile([128, KC, SC], bf16, tag="qT")
            kT = pp.tile([128, KC, T], bf16, tag="kT")
            vA = pp.tile([128, NT, H * 65], bf16, tag="vA")

            # trailing ones column for each head's fused rowsum
            for h in range(H):
                nc.vector.memset(vA[:, :, h * 65 + 64 : h * 65 + 65], 1)

            # ---- DMA schedule (one queue, ordered by first use) ----
            def dma_w(dst, src):
                nc.sync.dma_start(
                    out=dst[:], in_=src.ap().rearrange("(c p) e -> p c e", p=128)
                )

            def dma_x(dst, src, c2, width=512):
                sl = slice(c2 * width, (c2 + 1) * width)
                nc.sync.dma_start(
                    out=dst[:, :, sl],
                    in_=src.ap().rearrange("(c p) s -> p c s", p=128)[:, :, sl],
                )

            mtiles = []

            def dma_m(k):  # mask chunk of 2 t-blocks
                mt = mst.tile([128, 2, SC], bf16, tag="m")
                nc.sync.dma_start(
                    out=mt[:],
                    in_=mask_d.ap().rearrange("(c p) s -> p c s", p=128)[
                        :, 2 * k : 2 * k + 2, :
                    ],
                )
                mtiles.append(mt)

            dma_w(wq, wq_d)
            dma_x(xq, xq_d, 0)
            dma_m(0)
            dma_w(wk, wk_d)
            dma_x(xk, xk_d, 0)
            dma_w(wv, wv_d)
            dma_x(xv, xv_d, 0)
            dma_m(1)
            dma_x(xq, xq_d, 1)
            dma_m(2)
            dma_x(xk, xk_d, 1)
            dma_x(xv, xv_d, 1)
            dma_m(3)
            dma_m(4)
            dma_x(xk, xk_d, 2)
            dma_x(xv, xv_d, 2)
            dma_m(5)
            dma_m(6)
            dma_x(xk, xk_d, 3)
            dma_x(xv, xv_d, 3)
            dma_m(7)
            nc.sync.dma_start(
                out=wo[:], in_=wo_d.ap().rearrange("(h p) e -> p h e", p=64)
            )

            # ---- projection work units (drained during attention) ----
            def qT_unit(eo, c2):
                ps = pjp.tile([128, 512], f32, tag="pj")
                for kc in range(KC):
                    nc.tensor.matmul(
                        ps[:],
                        lhsT=wq[:, kc, eo * 128 : (eo + 1) * 128],
                        rhs=xq[:, kc, c2 * 512 : (c2 + 1) * 512],
                        start=(kc == 0),
                        stop=(kc == KC - 1),
                    )
                nc.vector.tensor_copy(
                    out=qT[:, eo, c2 * 512 : (c2 + 1) * 512], in_=ps[:]
                )

            def kT_unit(eo, c2):
                ps = pjp.tile([128, 512], f32, tag="pj")
                for kc in range(KC):
                    nc.tensor.matmul(
                        ps[:],
                        lhsT=wk[:, kc, eo * 128 : (eo + 1) * 128],
                        rhs=xk[:, kc, c2 * 512 : (c2 + 1) * 512],
                        start=(kc == 0),
                        stop=(kc == KC - 1),
                    )
                nc.vector.tensor_copy(
                    out=kT[:, eo, c2 * 512 : (c2 + 1) * 512], in_=ps[:]
                )

            def vN_unit(tb):
                ps = pjp.tile([128, 512], f32, tag="pj")
                for kc in range(KC):
                    nc.tensor.matmul(
                        ps[:],
                        lhsT=xv[:, kc, tb * 128 : (tb + 1) * 128],
                        rhs=wv[:, kc, :],
                        start=(kc == 0),
                        stop=(kc == KC - 1),
                    )
                nc.vector.tensor_copy(
                    out=vA[:, tb, :].rearrange("p (h c) -> p h c", h=H)[:, :, 0:64],
                    in_=ps[:].rearrange("p (h c) -> p h c", h=H),
                )

            ao_tiles = {}  # (sc, h) -> [65, 512] bf16 normalized attn out

            def op_unit(sc, blk):
                ps = pjp.tile([128, 512], f32, tag="pj")
                for h in range(H):
                    nc.tensor.matmul(
                        ps[:],
                        lhsT=ao_tiles[(sc, h)][:, blk * 128 : (blk + 1) * 128],
                        rhs=wo[:, h, :],
                        start=(h == 0),
                        stop=(h == H - 1),
                    )
                osb = osp.tile([128, 512], f32, tag="osb")
                nc.vector.tensor_copy(out=osb[:], in_=ps[:])
                r0 = sc * 512 + blk * 128
                nc.sync.dma_start(out=out_d.ap()[r0 : r0 + 128, :], in_=osb[:])

            # per-(group, iter) projection drain plan
            plan = [dict() for _ in range(NG)]
            plan[0] = {
                0: [("vN", 0)], 1: [("vN", 1), ("kT", 0, 1)], 2: [("vN", 2)],
                3: [("vN", 3)], 4: [("vN", 4), ("kT", 0, 2)], 5: [("vN", 5)],
                6: [("vN", 6)], 7: [("vN", 7), ("kT", 0, 3)], 8: [("vN", 8)],
                9: [("vN", 9)], 10: [("vN", 10)], 11: [("vN", 11)],
                12: [("vN", 12), ("kT", 1, 0)], 13: [("vN", 13), ("qT", 1, 0)],
                14: [("vN", 14), ("kT", 1, 1)], 15: [("vN", 15), ("kT", 1, 2)],
            }
            plan[1] = {
                0: [("kT", 1, 3)], 4: [("kT", 2, 0)], 6: [("qT", 2, 0)],
                8: [("kT", 2, 1)], 10: [("kT", 2, 2)], 12: [("kT", 2, 3)],
                14: [("kT", 3, 0)],
            }
            plan[2] = {
                0: [("qT", 3, 0)], 2: [("kT", 3, 1)], 4: [("kT", 3, 2)],
                6: [("kT", 3, 3)], 10: [("qT", 0, 1)], 12: [("qT", 1, 1)],
            }
            plan[3] = {2: [("qT", 2, 1)], 4: [("qT", 3, 1)]}
            plan[4] = {
                2: [("op", 0, 0)], 6: [("op", 0, 1)], 10: [("op", 0, 2)],
                14: [("op", 0, 3)],
            }

            def run_unit(u):
                if u[0] == "vN":
                    vN_unit(u[1])
                elif u[0] == "kT":
                    kT_unit(u[1], u[2])
                elif u[0] == "qT":
                    qT_unit(u[1], u[2])
                elif u[0] == "op":
                    op_unit(u[1], u[2])

            # ---- lead-in: just enough projection for group 0 ----
            qT_unit(0, 0)
            kT_unit(0, 0)

            # ---- attention: 8 groups of (sc-chunk, head-pair) ----
            mul_idx = 0
            prep_done = 0

            for gi in range(NG):
                sc, q = gi // 4, gi % 4
                h0 = 2 * q
                ssl = slice(sc * 512, (sc + 1) * 512)
                av = [
                    avp.tile([128, 512], f32, tag="av", name=f"av{j}")
                    for j in range(2)
                ]
                pend = []  # (due iter, tb, a2 tile)
                n_av = 0

                def emit_av(tb, a2, last):
                    for j in range(2):
                        nc.tensor.matmul(
                            av[j][0:65, :],
                            lhsT=vA[:, tb, (h0 + j) * 65 : (h0 + j + 1) * 65],
                            rhs=a2[:, j * 512 : (j + 1) * 512],
                            start=(n_av == 0),
                            stop=last,
                            skip_group_check=True,
                        )

                for t in range(NT):
                    # QK pair (row-quadrant packed, concurrent)
                    sp = spp.tile([128, 1024], f32, tag="sp")
                    for j in range(2):
                        jsl = slice(j * 64, (j + 1) * 64)
                        nc.tensor.matmul(
                            sp[:, j * 512 : (j + 1) * 512],
                            lhsT=kT[jsl, q, t * 128 : (t + 1) * 128],
                            rhs=qT[jsl, q, ssl],
                            start=True,
                            stop=True,
                            tile_position=(j * 64, 0),
                        )
                    # exp(mask) prepass, just-in-time within group 0
                    if gi == 0 and t % 2 == 0 and prep_done < NT // 2:
                        k = t // 2
                        nc.scalar.activation(
                            g[:, 2 * k : 2 * k + 2, :], mtiles[k][:], Exp
                        )
                        prep_done += 1
                    # exp(scores)
                    et = etp.tile([128, 1024], bf16, tag="et")
                    nc.scalar.activation(et[:], sp[:], Exp)
                    # A = exp(scores) * exp(mask), mask factor doubled across
                    # the two heads via a stride-0 broadcast
                    a2 = a2p.tile([128, 1024], bf16, tag="a2")
                    gb = g[:, t, ssl].unsqueeze(1).broadcast_to([128, 2, 512])
                    on_gps = t in GPS_T
                    eng = nc.gpsimd if on_gps else nc.vector
                    eng.tensor_tensor(
                        out=a2[:].rearrange("p (a b) -> p a b", a=2),
                        in0=et[:].rearrange("p (a b) -> p a b", a=2),
                        in1=gb,
                        op=Mult,
                    )
                    mul_idx += 1
                    pend.append((t + (GPS_LAG if on_gps else AV_LAG), t, a2))
                    # lagged AV so the PE never head-of-line blocks on exp/mul
                    for entry in sorted(pend):
                        if entry[0] <= t:
                            pend.remove(entry)
                            emit_av(entry[1], entry[2], False)
                            n_av += 1
                    # drain projection work
                    for u in plan[gi].get(t, ()):
                        run_unit(u)
                for i_, entry in enumerate(sorted(pend)):
                    emit_av(entry[1], entry[2], i_ == len(pend) - 1)
                    n_av += 1

                # ---- epilogue: normalize by the fused rowsum (row 64) ----
                for j in range(2):
                    rr = nrm.tile([96, 512], f32, tag="rr")
                    nc.vector.tensor_copy(out=rr[64:65, :], in_=av[j][64:65, :])
                    # rowsum row 64 -> rows 0:32 (cross-quadrant shuffle);
                    # all DVE ops must stay lane-aligned, so recip maps row 0
                    # -> row 0 of a fresh tile, then broadcast to 64 rows
                    nc.vector.stream_shuffle(rr[0:32, :], rr[64:96, :], [0] * 32)
                    r1 = nrm.tile([1, 512], f32, tag="r1")
                    nc.vector.reciprocal_approx_fast(out=r1[:], in_=rr[0:1, :])
                    rb = nrm.tile([64, 512], f32, tag="rb")
                    nc.gpsimd.partition_broadcast(rb[:], r1[:])
                    ao = aop.tile([64, 512], bf16, tag="ao")
                    nc.vector.tensor_tensor(
                        out=ao[:], in0=av[j][0:64, :], in1=rb[:], op=Mult
                    )
                    ao_tiles[(sc, h0 + j)] = ao

            # ---- tail: out-proj for sc=1 ----
            for blk in range(4):
                op_unit(1, blk)

    nc.compile()
    return nc


def _get_compiled():
    global _compiled
    if _compiled is None:
        _compiled = _build()
    return _compiled


def _prep_in_maps(query, key, value, attn_mask, in_proj_weight):
    import ml_dtypes

    bf = ml_dtypes.bfloat16
    q_t = np.ascontiguousarray(query.transpose(1, 2, 0).astype(bf))   # [B, E, S]
    k_t = np.ascontiguousarray(key.transpose(1, 2, 0).astype(bf))
    v_t = np.ascontiguousarray(value.transpose(1, 2, 0).astype(bf))
    m_t = np.ascontiguousarray(attn_mask.transpose(0, 2, 1).astype(bf))  # [B,T,S]
    # 1/sqrt(D) = 1/8 folded into Wq -- exact in fp32 (power of two)
    wq_t = np.ascontiguousarray((in_proj_weight[0:E] * 0.125).T.astype(bf))
    wk_t = np.ascontiguousarray(in_proj_weight[E : 2 * E].T.astype(bf))
    wv_t = np.ascontiguousarray(in_proj_weight[2 * E : 3 * E].T.astype(bf))
    in_maps = []
    for c in range(NCORES):
        b, hf = c // 2, c % 2
        sl = slice(hf * SC, (hf + 1) * SC)
        in_maps.append(
            {
                "xq_t": np.ascontiguousarray(q_t[b][:, sl]),
                "xk_t": k_t[b],
                "xv_t": v_t[b],
                "mask_t": np.ascontiguousarray(m_t[b][:, sl]),
                "wq_t": wq_t,
                "wk_t": wk_t,
                "wv_t": wv_t,
            }
        )
    return in_maps


def kernel(
    query,
    key,
    value,
    attn_mask,
    in_proj_weight,
    in_proj_bias,
    out_proj_weight,
    out_proj_bias,
):
    from concourse.bass_utils import run_bass_kernel_spmd

    query = np.asarray(query, np.float32)
    key = np.asarray(key, np.float32)
    value = np.asarray(value, np.float32)
    attn_mask = np.asarray(attn_mask, np.float32)
    in_proj_weight = np.asarray(in_proj_weight, np.float32)
    out_proj_weight = np.asarray(out_proj_weight, np.float32)
    out_proj_bias = np.asarray(out_proj_bias, np.float32)

    nc = _get_compiled()
    in_maps = _prep_in_maps(query, key, value, attn_mask, in_proj_weight)
    import ml_dtypes

    wo_t = np.ascontiguousarray(out_proj_weight.T.astype(ml_dtypes.bfloat16))
    for m in in_maps:
        m["wo_t"] = wo_t

    res = run_bass_kernel_spmd(nc, in_maps, core_ids=list(range(NCORES)))

    out = np.empty((S, B, E), np.float32)
    for c in range(NCORES):
        b, hf = c // 2, c % 2
        out[hf * SC : (hf + 1) * SC, b, :] = res.results[c]["out"]
    # out_proj_bias is zeros per the problem spec; adding it on the host is
    # exact. (in_proj biases are also zeros and are not applied on-device.)
    out += out_proj_bias[None, None, :]
    return out



# revision 2
# speedup vs baseline: 1.1618x; 1.1618x over previous
"""Trainium2 Bass kernel for nn_ContentMultiheadAttention_523986010170.

Full (unsharded) inputs in, full output out. Internally shards across 8
NeuronCores: core c handles batch b = c//2 and query-row half c%2 (1024 of
2048 rows), computing all 8 heads for its slice. Outputs are disjoint
[1024, 512] blocks of the [S, B, E] result, gathered on the host.

v3 design (ACT-paced, decoupled epilogue): the scalar engine's 128 score
exps + 8 mask exps (~163us busy) are the hard floor; everything else is
scheduled to keep that stream gap-free and the PE warm (HAM K=8/8):

  - 2-head groups x 16 t-blocks: one [128,1024] f32 score psum + one exp
    per iteration.  PSUM = 4 banks (scores x2) + 2 (AV accum) + 2 (proj).
  - rowsum folded into the AV matmul: lhsT = [v_h | ones] (M=65), so row
    64 of the AV accumulator is the softmax denominator.
  - A = exp(scores) * exp(mask): single [128,1024] multiply per iteration
    with the mask factor free-dim-broadcast; split DVE(11/16)/GPSIMD(5/16).
  - group epilogue (normalize: copy/shuffle/recip/partition_broadcast/mul)
    is NOT emitted at the group boundary: its 6 steps are spread over the
    first 4 iterations of the NEXT group, so no engine queue head-of-line
    blocks and the PSUM AV accumulators recycle just in time (lazy alloc).
  - PE + ACT warmup: 8 dummy matmuls flip the HAM clock gate to 2.4 GHz
    during the first DMAs; a dummy exp pulls the ACT table load (~2.7us)
    off the critical path.
  - DMAs ordered by first use (wq,xq0,wk,xk0 first) so the first QK lands
    at ~6us; q/k/v projections drained through a per-iteration work plan.

Host-side work is limited to layout (transpose/slice/concat), the exact
power-of-two weight prescale, and adding out_proj_bias (a zero vector per
the problem spec; in_proj biases are likewise zero and are not applied).
"""

import numpy as np

S, B, E = 2048, 4, 512
H, D = 8, 64
NCORES = 8
SC = S // 2          # query rows per core
T = S                # key rows (full)
NT = T // 128        # t-blocks of 128
KC = E // 128        # contraction chunks for projections
NG = 8               # attention groups: (sc-chunk, head-pair)
AV_LAG = 3           # AV emission lag (iters) behind QK for DVE-mul tiles
GPS_LAG = 7          # lag for gpsimd-mul tiles (hides Q7 latency)
GPS_T = (2, 5, 8, 11, 14)   # iterations whose A-multiply goes to gpsimd

_compiled = None


def _build():
    import concourse.bacc as bacc
    import concourse.mybir as mybir
    import concourse.tile as tile

    f32 = mybir.dt.float32
    bf16 = mybir.dt.bfloat16
    Exp = mybir.ActivationFunctionType.Exp
    Mult = mybir.AluOpType.mult

    nc = bacc.Bacc("TRN2", target_bir_lowering=False, debug=False)

    xq_d = nc.dram_tensor("xq_t", [E, SC], bf16, kind="ExternalInput")
    xk_d = nc.dram_tensor("xk_t", [E, T], bf16, kind="ExternalInput")
    xv_d = nc.dram_tensor("xv_t", [E, T], bf16, kind="ExternalInput")
    mask_d = nc.dram_tensor("mask_t", [T, SC], bf16, kind="ExternalInput")
    wq_d = nc.dram_tensor("wq_t", [E, E], bf16, kind="ExternalInput")
    wk_d = nc.dram_tensor("wk_t", [E, E], bf16, kind="ExternalInput")
    wv_d = nc.dram_tensor("wv_t", [E, E], bf16, kind="ExternalInput")
    wo_d = nc.dram_tensor("wo_t", [E, E], bf16, kind="ExternalInput")
    out_d = nc.dram_tensor("out", [SC, E], f32, kind="ExternalOutput")

    with tile.TileContext(nc) as tc:
        with (
            tc.tile_pool(name="persist", bufs=1) as pp,
            tc.tile_pool(name="mstage", bufs=2) as mst,
            tc.tile_pool(name="et", bufs=6) as etp,
            tc.tile_pool(name="a2", bufs=10) as a2p,
            tc.tile_pool(name="ao", bufs=16) as aop,
            tc.tile_pool(name="norm", bufs=2) as nrm,
            tc.tile_pool(name="osb", bufs=3) as osp,
            tc.tile_pool(name="sp", bufs=2, space="PSUM") as spp,
            tc.tile_pool(name="av", bufs=2, space="PSUM") as avp,
            tc.tile_pool(name="pj", bufs=2, space="PSUM") as pjp,
        ):
            # ---- persistent SBUF tensors ----
            wq = pp.tile([128, KC, E], bf16, tag="wq")
            wk = pp.tile([128, KC, E], bf16, tag="wk")
            wv = pp.tile([128, KC, E], bf16, tag="wv")
            wo = pp.tile([64, H, E], bf16, tag="wo")  # head-major, base-0 rows
            xq = pp.tile([128, KC, SC], bf16, tag="xq")
            xk = pp.tile([128, KC, T], bf16, tag="xk")
            xv = pp.tile([128, KC, T], bf16, tag="xv")
            g = pp.tile([128, NT, SC], bf16, tag="g")
            qT = pp.tile([128, KC, SC], bf16, tag="qT")
            kT = pp.tile([128, KC, T], bf16, tag="kT")
            vA = pp.tile([128, NT, H * 65], bf16, tag="vA")

            # ---- engine warmup: flip HAM to K=8/8 and pull the ACT ----
            # ---- exp-table load off the critical path, during lead-in DMAs
            ws = pp.tile([128, 512], bf16, tag="ws")
            nc.vector.memset(ws[:], 0)
            wact = pp.tile([1, 16], f32, tag="wact")
            nc.scalar.activation(wact[:], ws[0:1, 0:16], Exp)
            wps = pjp.tile([128, 512], f32, tag="pj")
            for _ in range(8):
                nc.tensor.matmul(
                    wps[:], lhsT=ws[:, 0:128], rhs=ws[:], start=True, stop=True
                )

            # trailing ones column for each head's fused rowsum
            for h in range(H):
                nc.vector.memset(vA[:, :, h * 65 + 64 : h * 65 + 65], 1)

            # ---- DMA schedule (one queue, ordered by first use) ----
            def dma_w(dst, src):
                nc.sync.dma_start(
                    out=dst[:], in_=src.ap().rearrange("(c p) e -> p c e", p=128)
                )

            def dma_x(dst, src, c2, width=512):
                sl = slice(c2 * width, (c2 + 1) * width)
                nc.sync.dma_start(
                    out=dst[:, :, sl],
                    in_=src.ap().rearrange("(c p) s -> p c s", p=128)[:, :, sl],
                )

            mtiles = []

            def dma_m(k):  # mask chunk of 2 t-blocks
                mt = mst.tile([128, 2, SC], bf16, tag="m")
                nc.sync.dma_start(
                    out=mt[:],
                    in_=mask_d.ap().rearrange("(c p) s -> p c s", p=128)[
                        :, 2 * k : 2 * k + 2, :
                    ],
                )
                mtiles.append(mt)

            dma_w(wq, wq_d)
            dma_x(xq, xq_d, 0)
            dma_w(wk, wk_d)
            dma_x(xk, xk_d, 0)
            dma_m(0)
            dma_w(wv, wv_d)
            dma_x(xv, xv_d, 0)
            dma_m(1)
            dma_x(xq, xq_d, 1)
            dma_m(2)
            dma_x(xk, xk_d, 1)
            dma_x(xv, xv_d, 1)
            dma_m(3)
            dma_m(4)
            dma_x(xk, xk_d, 2)
            dma_x(xv, xv_d, 2)
            dma_m(5)
            dma_m(6)
            dma_x(xk, xk_d, 3)
            dma_x(xv, xv_d, 3)
            dma_m(7)
            nc.sync.dma_start(
                out=wo[:], in_=wo_d.ap().rearrange("(h p) e -> p h e", p=64)
            )

            # ---- projection work units (drained during attention) ----
            def qT_unit(eo, c2):
                ps = pjp.tile([128, 512], f32, tag="pj")
                for kc in range(KC):
                    nc.tensor.matmul(
                        ps[:],
                        lhsT=wq[:, kc, eo * 128 : (eo + 1) * 128],
                        rhs=xq[:, kc, c2 * 512 : (c2 + 1) * 512],
                        start=(kc == 0),
                        stop=(kc == KC - 1),
                    )
                nc.vector.tensor_copy(
                    out=qT[:, eo, c2 * 512 : (c2 + 1) * 512], in_=ps[:]
                )

            def kT_unit(eo, c2):
                ps = pjp.tile([128, 512], f32, tag="pj")
                for kc in range(KC):
                    nc.tensor.matmul(
                        ps[:],
                        lhsT=wk[:, kc, eo * 128 : (eo + 1) * 128],
                        rhs=xk[:, kc, c2 * 512 : (c2 + 1) * 512],
                        start=(kc == 0),
                        stop=(kc == KC - 1),
                    )
                nc.vector.tensor_copy(
                    out=kT[:, eo, c2 * 512 : (c2 + 1) * 512], in_=ps[:]
                )

            def vN_unit(tb):
                ps = pjp.tile([128, 512], f32, tag="pj")
                for kc in range(KC):
                    nc.tensor.matmul(
                        ps[:],
                        lhsT=xv[:, kc, tb * 128 : (tb + 1) * 128],
                        rhs=wv[:, kc, :],
                        start=(kc == 0),
                        stop=(kc == KC - 1),
                    )
                nc.vector.tensor_copy(
                    out=vA[:, tb, :].rearrange("p (h c) -> p h c", h=H)[:, :, 0:64],
                    in_=ps[:].rearrange("p (h c) -> p h c", h=H),
                )

            ao_tiles = {}  # (sc, h) -> [65, 512] bf16 normalized attn out

            def op_unit(sc, blk):
                ps = pjp.tile([128, 512], f32, tag="pj")
                for h in range(H):
                    nc.tensor.matmul(
                        ps[:],
                        lhsT=ao_tiles[(sc, h)][:, blk * 128 : (blk + 1) * 128],
                        rhs=wo[:, h, :],
                        start=(h == 0),
                        stop=(h == H - 1),
                    )
                osb = osp.tile([128, 512], f32, tag="osb")
                nc.vector.tensor_copy(out=osb[:], in_=ps[:])
                r0 = sc * 512 + blk * 128
                nc.sync.dma_start(out=out_d.ap()[r0 : r0 + 128, :], in_=osb[:])

            # per-(group, iter) projection drain plan
            plan = [dict() for _ in range(NG)]
            plan[0] = {
                0: [("vN", 0)], 1: [("vN", 1), ("kT", 0, 1)], 2: [("vN", 2)],
                3: [("vN", 3)], 4: [("vN", 4), ("kT", 0, 2)], 5: [("vN", 5)],
                6: [("vN", 6)], 7: [("vN", 7), ("kT", 0, 3)], 8: [("vN", 8)],
                9: [("vN", 9)], 10: [("vN", 10)], 11: [("vN", 11)],
                12: [("vN", 12), ("kT", 1, 0)], 13: [("vN", 13), ("qT", 1, 0)],
                14: [("vN", 14), ("kT", 1, 1)], 15: [("vN", 15), ("kT", 1, 2)],
            }
            plan[1] = {
                0: [("kT", 1, 3)], 4: [("kT", 2, 0)], 6: [("qT", 2, 0)],
                8: [("kT", 2, 1)], 10: [("kT", 2, 2)], 12: [("kT", 2, 3)],
                14: [("kT", 3, 0)],
            }
            plan[2] = {
                0: [("qT", 3, 0)], 2: [("kT", 3, 1)], 4: [("kT", 3, 2)],
                6: [("kT", 3, 3)], 10: [("qT", 0, 1)], 12: [("qT", 1, 1)],
            }
            plan[3] = {2: [("qT", 2, 1)], 4: [("qT", 3, 1)]}
            plan[4] = {
                4: [("op", 0, 0)], 7: [("op", 0, 1)], 10: [("op", 0, 2)],
                13: [("op", 0, 3)],
            }

            def run_unit(u):
                if u[0] == "vN":
                    vN_unit(u[1])
                elif u[0] == "kT":
                    kT_unit(u[1], u[2])
                elif u[0] == "qT":
                    qT_unit(u[1], u[2])
                elif u[0] == "op":
                    op_unit(u[1], u[2])

            # ---- decoupled group epilogue (normalize by fused rowsum) ----
            # Emitted as 6 steps spread over the first iterations of the
            # NEXT group so no engine queue head-of-line blocks on it.
            def epi_steps(av, sc, h0):
                state = {}

                def s_shuf(j):
                    def run():
                        rr = nrm.tile([96, 512], f32, tag="rr")
                        nc.vector.tensor_copy(out=rr[64:65, :], in_=av[j][64:65, :])
                        # rowsum row 64 -> rows 0:32 (cross-quadrant shuffle);
                        # DVE ops stay lane-aligned, so recip maps row 0 -> row
                        # 0 of a fresh tile, then broadcast to 64 rows
                        nc.vector.stream_shuffle(rr[0:32, :], rr[64:96, :], [0] * 32)
                        state[("rr", j)] = rr

                    return run

                def s_recip(j):
                    def run():
                        r1 = nrm.tile([1, 512], f32, tag="r1")
                        nc.vector.reciprocal_approx_fast(
                            out=r1[:], in_=state[("rr", j)][0:1, :]
                        )
                        rb = nrm.tile([64, 512], f32, tag="rb")
                        nc.gpsimd.partition_broadcast(rb[:], r1[:])
                        state[("rb", j)] = rb

                    return run

                def s_ao(j):
                    def run():
                        ao = aop.tile([64, 512], bf16, tag="ao")
                        nc.vector.tensor_tensor(
                            out=ao[:],
                            in0=av[j][0:64, :],
                            in1=state[("rb", j)],
                            op=Mult,
                        )
                        ao_tiles[(sc, h0 + j)] = ao

                    return run

                return [
                    s_shuf(0), s_recip(0), s_shuf(1),
                    s_ao(0), s_recip(1), s_ao(1),
                ]

            # ---- lead-in: just enough projection for group 0 ----
            qT_unit(0, 0)
            kT_unit(0, 0)

            # ---- attention: 8 groups of (sc-chunk, head-pair) ----
            prep_done = 0
            pend_epi = []  # epilogue steps of the previous group

            for gi in range(NG):
                sc, q = gi // 4, gi % 4
                h0 = 2 * q
                ssl = slice(sc * 512, (sc + 1) * 512)
                av = [None, None]  # lazy PSUM alloc (after prev group frees)
                pend = []  # (due iter, tb, a2 tile)
                n_av = 0

                def emit_av(tb, a2, last):
                    nonlocal n_av
                    if av[0] is None:
                        for j in range(2):
                            av[j] = avp.tile(
                                [128, 512], f32, tag="av", name=f"av{j}"
                            )
                    for j in range(2):
                        nc.tensor.matmul(
                            av[j][0:65, :],
                            lhsT=vA[:, tb, (h0 + j) * 65 : (h0 + j + 1) * 65],
                            rhs=a2[:, j * 512 : (j + 1) * 512],
                            start=(n_av == 0),
                            stop=last,
                            skip_group_check=True,
                        )
                    n_av += 1

                for t in range(NT):
                    # previous group's epilogue, two steps per iteration
                    for _ in range(2):
                        if pend_epi:
                            pend_epi.pop(0)()
                    # QK pair (row-quadrant packed, concurrent)
                    sp = spp.tile([128, 1024], f32, tag="sp")
                    for j in range(2):
                        jsl = slice(j * 64, (j + 1) * 64)
                        nc.tensor.matmul(
                            sp[:, j * 512 : (j + 1) * 512],
                            lhsT=kT[jsl, q, t * 128 : (t + 1) * 128],
                            rhs=qT[jsl, q, ssl],
                            start=True,
                            stop=True,
                            tile_position=(j * 64, 0),
                        )
                    # exp(scores)
                    et = etp.tile([128, 1024], bf16, tag="et")
                    nc.scalar.activation(et[:], sp[:], Exp)
                    # exp(mask) prepass, just-in-time within group 0 (after
                    # the score exp so it never head-of-line blocks it)
                    if gi == 0 and t % 2 == 0 and prep_done < NT // 2:
                        k = t // 2
                        nc.scalar.activation(
                            g[:, 2 * k : 2 * k + 2, :], mtiles[k][:], Exp
                        )
                        prep_done += 1
                    # A = exp(scores) * exp(mask), mask factor doubled across
                    # the two heads via a stride-0 broadcast
                    a2 = a2p.tile([128, 1024], bf16, tag="a2")
                    gb = g[:, t, ssl].unsqueeze(1).broadcast_to([128, 2, 512])
                    on_gps = t in GPS_T
                    eng = nc.gpsimd if on_gps else nc.vector
                    eng.tensor_tensor(
                        out=a2[:].rearrange("p (a b) -> p a b", a=2),
                        in0=et[:].rearrange("p (a b) -> p a b", a=2),
                        in1=gb,
                        op=Mult,
                    )
                    pend.append((t + (GPS_LAG if on_gps else AV_LAG), t, a2))
                    # lagged AV so the PE never head-of-line blocks on exp/mul
                    for entry in sorted(pend):
                        if entry[0] <= t:
                            pend.remove(entry)
                            emit_av(entry[1], entry[2], False)
                    # drain projection work
                    for u in plan[gi].get(t, ()):
                        run_unit(u)
                while pend_epi:  # safety: never carry into epi_steps rebuild
                    pend_epi.pop(0)()
                for i_, entry in enumerate(sorted(pend)):
                    emit_av(entry[1], entry[2], i_ == len(pend) - 1)

                pend_epi = epi_steps(av, sc, h0)

            # ---- tail: last group's epilogue, then out-proj for sc=1 ----
            while pend_epi:
                pend_epi.pop(0)()
            for blk in range(4):
                op_unit(1, blk)

    nc.compile()
    return nc


def _get_compiled():
    global _compiled
    if _compiled is None:
        _compiled = _build()
    return _compiled


def _prep_in_maps(query, key, value, attn_mask, in_proj_weight):
    import ml_dtypes

    bf = ml_dtypes.bfloat16
    q_t = np.ascontiguousarray(query.transpose(1, 2, 0).astype(bf))   # [B, E, S]
    k_t = np.ascontiguousarray(key.transpose(1, 2, 0).astype(bf))
    v_t = np.ascontiguousarray(value.transpose(1, 2, 0).astype(bf))
    m_t = np.ascontiguousarray(attn_mask.transpose(0, 2, 1).astype(bf))  # [B,T,S]
    # 1/sqrt(D) = 1/8 folded into Wq -- exact in fp32 (power of two)
    wq_t = np.ascontiguousarray((in_proj_weight[0:E] * 0.125).T.astype(bf))
    wk_t = np.ascontiguousarray(in_proj_weight[E : 2 * E].T.astype(bf))
    wv_t = np.ascontiguousarray(in_proj_weight[2 * E : 3 * E].T.astype(bf))
    in_maps = []
    for c in range(NCORES):
        b, hf = c // 2, c % 2
        sl = slice(hf * SC, (hf + 1) * SC)
        in_maps.append(
            {
                "xq_t": np.ascontiguousarray(q_t[b][:, sl]),
                "xk_t": k_t[b],
                "xv_t": v_t[b],
                "mask_t": np.ascontiguousarray(m_t[b][:, sl]),
                "wq_t": wq_t,
                "wk_t": wk_t,
                "wv_t": wv_t,
            }
        )
    return in_maps


def kernel(
    query,
    key,
    value,
    attn_mask,
    in_proj_weight,
    in_proj_bias,
    out_proj_weight,
    out_proj_bias,
):
    from concourse.bass_utils import run_bass_kernel_spmd

    query = np.asarray(query, np.float32)
    key = np.asarray(key, np.float32)
    value = np.asarray(value, np.float32)
    attn_mask = np.asarray(attn_mask, np.float32)
    in_proj_weight = np.asarray(in_proj_weight, np.float32)
    out_proj_weight = np.asarray(out_proj_weight, np.float32)
    out_proj_bias = np.asarray(out_proj_bias, np.float32)

    nc = _get_compiled()
    in_maps = _prep_in_maps(query, key, value, attn_mask, in_proj_weight)
    import ml_dtypes

    wo_t = np.ascontiguousarray(out_proj_weight.T.astype(ml_dtypes.bfloat16))
    for m in in_maps:
        m["wo_t"] = wo_t

    res = run_bass_kernel_spmd(nc, in_maps, core_ids=list(range(NCORES)))

    out = np.empty((S, B, E), np.float32)
    for c in range(NCORES):
        b, hf = c // 2, c % 2
        out[hf * SC : (hf + 1) * SC, b, :] = res.results[c]["out"]
    # out_proj_bias is zeros per the problem spec; adding it on the host is
    # exact. (in_proj biases are also zeros and are not applied on-device.)
    out += out_proj_bias[None, None, :]
    return out
